# revision 16
# baseline (speedup 1.0000x reference)
"""Trainium2 Bass kernel for a single transformer decoder layer
(B=2, S=2048, E=2048, 16 heads, FFN 4x, causal attention, exact gelu,
two layernorms), distributed over 8 NeuronCores.

Sharding:
  - QKV + attention: tensor-parallel over heads (2 heads/core), zero comm.
  - One AllToAll exchanges ctx slices ([head-slice, all tokens] ->
    [all heads, 512-token slice]); each core then runs the fc projection
    with the full Wfc plus LN1 + FFN (full W1/W2) + LN2 on its own
    512-token slice. Host concatenates the 8 output slices.

Everything on-chip stays transposed ([feature, token]) so biases and
layernorm gains are per-partition ops and no transposes are needed.
Matmuls run in float32r (~13-bit mantissa, bf16 speed at N>=512).
"""
import functools
import math

import numpy as np

import concourse.bacc as bacc
import concourse.bass as bass
import concourse.mybir as mybir
import concourse.tile as tile
from concourse.bass_utils import run_bass_kernel_spmd

N_CORES = 8
P = 128
B, S, E = 2, 2048, 2048
T = B * S                   # 4096 tokens
NH, HD = 16, 128
FF = 4 * E                  # 8192
KE = E // P                 # 16 contraction chunks
CPC = 2 * HD                # 256 head-dim columns per core
TBLK = T // N_CORES         # 512 tokens per core after the all-to-all
EPS = 1e-5

F32 = mybir.dt.float32
F32R = mybir.dt.float32r

Identity = mybir.ActivationFunctionType.Identity
Copy = mybir.ActivationFunctionType.Copy
Exp = mybir.ActivationFunctionType.Exp
Gelu = mybir.ActivationFunctionType.Gelu
Sqrt = mybir.ActivationFunctionType.Sqrt
ADD = mybir.AluOpType.add
MULT = mybir.AluOpType.mult
SUB = mybir.AluOpType.subtract


def _ln_finish(nc, pool, psums, x_t, ones, grows_d, gi, g_t, be_t, eps_t,
               out_t, mu_ps, sq_ps, tag, chunk_done=None):
    """Finish a layernorm whose \u03a3x and \u03a3x\u00b2 already sit in mu_ps/sq_ps
    [1,512] psums. x_t [128, KE, 512] f32r -> out_t.
    Apply is 2 DVE passes/chunk: out = (x*g)*bcast(rstd) + be - g\u2297(\u03bc*rstd)."""
    grows = pool.tile([1, KE * P], F32R, tag=f"{tag}_grows", bufs=1,
                      name=f"{tag}_grows")
    nc.sync.dma_start(grows[:], grows_d[:, gi * KE * P:(gi + 1) * KE * P])
    mu_sb = pool.tile([1, 512], F32, tag=f"{tag}_musb", bufs=1, name=f"{tag}_musb")
    nc.scalar.activation(mu_sb[:], mu_ps[:], Copy, scale=1.0 / E)
    m2_sb = pool.tile([1, 512], F32, tag=f"{tag}_m2sb", bufs=1, name=f"{tag}_m2sb")
    nc.scalar.activation(m2_sb[:], sq_ps[:], Copy, scale=1.0 / E)
    var = pool.tile([1, 512], F32, tag=f"{tag}_var", bufs=1, name=f"{tag}_var")
    nc.vector.tensor_mul(var[:], mu_sb[:], mu_sb[:])
    nc.vector.tensor_sub(var[:], m2_sb[:], var[:])
    std = pool.tile([1, 512], F32, tag=f"{tag}_std", bufs=1, name=f"{tag}_std")
    nc.scalar.activation(std[:], var[:], Sqrt, bias=eps_t[:])
    rstd = pool.tile([1, 512], F32R, tag=f"{tag}_rstd", bufs=1, name=f"{tag}_rstd")
    with nc.allow_low_precision(reason="f32r rstd feeds f32r broadcast matmul"):
        nc.vector.reciprocal(rstd[:], std[:])
    msr = pool.tile([1, 512], F32R, tag=f"{tag}_msr", bufs=1, name=f"{tag}_msr")
    nc.vector.tensor_mul(msr[:], mu_sb[:], rstd[:])
    rbc = psums.tile([P, 512], F32, tag=f"{tag}_rbc", bufs=1, name=f"{tag}_rbc")
    nc.tensor.matmul(rbc[:], ones[0:1, :], rstd[:], start=True, stop=True)
    for k in range(KE):
        mbcg = psums.tile([P, 512], F32, tag=f"{tag}_mbcg", bufs=2,
                          name=f"{tag}_mbcg")
        nc.tensor.matmul(
            mbcg[:], grows[0:1, k * P:(k + 1) * P],
            msr[:], start=True, stop=True)
        t1 = pool.tile([P, 512], F32, tag=f"{tag}_t1", bufs=2, name=f"{tag}_t1")
        nc.vector.scalar_tensor_tensor(
            t1[:], x_t[:, k, :], g_t[:, k:k + 1], rbc[:], MULT, MULT)
        nc.vector.scalar_tensor_tensor(
            out_t[:, k, :], t1[:], be_t[:, k:k + 1], mbcg[:], ADD, SUB)
        if chunk_done is not None:
            chunk_done(k)
    # (grows row gi*KE+k holds g[k*128:(k+1)*128] so mbcg = g_e * (mu*rstd)_t)


def _build_program():
    nc = bacc.Bacc("TRN2", target_bir_lowering=False, debug=False,
                   num_devices=N_CORES)

    # ---- per-core external inputs ----
    embT_d = nc.dram_tensor("embT", [E, T], F32R, kind="ExternalInput")
    embres_d = nc.dram_tensor("embres", [P, KE * TBLK], F32, kind="ExternalInput")
    wq_d = nc.dram_tensor("wq", [P, KE * CPC], F32R, kind="ExternalInput")
    wk_d = nc.dram_tensor("wk", [P, KE * CPC], F32R, kind="ExternalInput")
    wv_d = nc.dram_tensor("wv", [P, KE * CPC], F32R, kind="ExternalInput")
    bqk_d = nc.dram_tensor("bqk", [P, 4], F32, kind="ExternalInput")  # bq|bk chunks
    bvbc_d = nc.dram_tensor("bvbc", [P, CPC], F32, kind="ExternalInput")
    wfc_d = nc.dram_tensor("wfc", [16, P, KE * P], F32R, kind="ExternalInput")
    vecs_d = nc.dram_tensor("vecs", [P, 6 * KE], F32, kind="ExternalInput")
    # vecs: [bfc | g1 | be1 | b2 | g2 | be2] each [P, KE]
    w1_d = nc.dram_tensor("w1", [64, P, KE * P], F32R, kind="ExternalInput")
    b1_d = nc.dram_tensor("b1", [P, 64], F32, kind="ExternalInput")
    w2_d = nc.dram_tensor("w2", [4, 16, P, 16 * P], F32R, kind="ExternalInput")
    mask_d = nc.dram_tensor("maskT", [P, 4 * 512], F32R, kind="ExternalInput")
    ones_d = nc.dram_tensor("onesblk", [P, P], F32R, kind="ExternalInput")
    eye_d = nc.dram_tensor("eyeblk", [P, P], F32R, kind="ExternalInput")
    grows_d = nc.dram_tensor("grows", [1, 2 * KE * P], F32R, kind="ExternalInput")

    out_d = nc.dram_tensor("outp", [P, KE, TBLK], F32, kind="ExternalOutput")

    # ---- internal DRAM ----
    qT_d = nc.dram_tensor("qT_i", [CPC, T], F32R, kind="Internal")
    kT_d = nc.dram_tensor("kT_i", [CPC, T], F32R, kind="Internal")
    v_d = nc.dram_tensor("v_i", [T, CPC], F32R, kind="Internal")
    HB = TBLK // 2   # 256-token half-block
    a2a0in_d = nc.dram_tensor("a2a0in_i", [N_CORES, CPC, HB], F32R, kind="Internal")
    a2a0out_d = nc.dram_tensor("a2a0out_i", [N_CORES, CPC, HB], F32R, kind="Internal")
    a2a1ain_d = nc.dram_tensor("a2a1ain_i", [N_CORES, P, HB], F32R, kind="Internal")
    a2a1aout_d = nc.dram_tensor("a2a1aout_i", [N_CORES, P, HB], F32R, kind="Internal")
    a2a1bin_d = nc.dram_tensor("a2a1bin_i", [N_CORES, P, HB], F32R, kind="Internal")
    a2a1bout_d = nc.dram_tensor("a2a1bout_i", [N_CORES, P, HB], F32R, kind="Internal")

    with tile.TileContext(nc) as tc:
        with (
            tc.tile_pool(name="const", bufs=1) as cpool,
            tc.tile_pool(name="persist", bufs=1) as ppool,
        ):
            ones = cpool.tile([P, P], F32R, name="ones")
            nc.sync.dma_start(ones[:], ones_d[:])
            eye = cpool.tile([P, P], F32R, name="eye")
            nc.sync.dma_start(eye[:], eye_d[:])
            mask_t = cpool.tile([P, 4, 512], F32R, name="mask_t")
            nc.sync.dma_start(mask_t[:], mask_d[:].rearrange("p (f t) -> p f t", f=4))
            bqk_t = cpool.tile([P, 4], F32, name="bqk_t")
            nc.sync.dma_start(bqk_t[:], bqk_d[:])
            bvbc_t = cpool.tile([P, CPC], F32, name="bvbc_t")
            nc.sync.dma_start(bvbc_t[:], bvbc_d[:])
            vecs_t = cpool.tile([P, 6, KE], F32, name="vecs_t")
            nc.sync.dma_start(vecs_t[:], vecs_d[:].rearrange("p (v k) -> p v k", v=6))
            b1_t = cpool.tile([P, 64], F32, name="b1_t")
            nc.sync.dma_start(b1_t[:], b1_d[:])
            eps_t = cpool.tile([1, 1], F32, name="eps_t")
            nc.vector.memset(eps_t[:], EPS)

            bfc_t = vecs_t[:, 0, :]
            g1_t = vecs_t[:, 1, :]
            be1_t = vecs_t[:, 2, :]
            b2_t = vecs_t[:, 3, :]
            g2_t = vecs_t[:, 4, :]
            be2_t = vecs_t[:, 5, :]

            old_t = ppool.tile([P, KE, TBLK], F32R, name="old_t")   # LN1 output
            y_sb = ppool.tile([P, KE, TBLK], F32, name="y_sb")      # FFN accum

            # ================= Phase Q: q/k/v projections =================
            with (
                tc.tile_pool(name="qw", bufs=1) as qw,
                tc.tile_pool(name="qio", bufs=2) as qio,
                tc.tile_pool(name="qps", bufs=1, space="PSUM") as qps,
            ):
                wq_t = qw.tile([P, KE, CPC], F32R, name="wq_t")
                nc.sync.dma_start(wq_t[:], wq_d[:].rearrange("p (k m) -> p k m", k=KE))
                wk_t = qw.tile([P, KE, CPC], F32R, name="wk_t")
                nc.sync.dma_start(wk_t[:], wk_d[:].rearrange("p (k m) -> p k m", k=KE))
                wv_t = qw.tile([P, KE, CPC], F32R, name="wv_t")
                nc.sync.dma_start(wv_t[:], wv_d[:].rearrange("p (k m) -> p k m", k=KE))

                for tb in range(8):
                    e_t = qio.tile([P, KE, 512], F32R, tag="emb", bufs=2, name="e_t")
                    nc.sync.dma_start(
                        e_t[:],
                        embT_d[:, tb * 512:(tb + 1) * 512]
                        .rearrange("(k p) t -> p k t", p=P),
                    )
                    for wi, (wt, dst) in enumerate(((wq_t, qT_d), (wk_t, kT_d))):
                        for hc in range(2):
                            pqk = qps.tile([P, 512], F32, tag="pqk", bufs=3, name="pqk")
                            for k in range(KE):
                                nc.tensor.matmul(
                                    pqk[:], wt[:, k, hc * P:(hc + 1) * P],
                                    e_t[:, k, :],
                                    start=(k == 0), stop=(k == KE - 1),
                                )
                            st = qio.tile([P, 512], F32R, tag="qkst", bufs=4, name="st")
                            nc.scalar.activation(st[:], pqk[:], Identity,
                                                 bias=bqk_t[:, 2 * wi + hc:2 * wi + hc + 1])
                            nc.sync.dma_start(
                                dst.ap()[hc * P:(hc + 1) * P, tb * 512:(tb + 1) * 512],
                                st[:])
                    for tt in range(4):
                        pv = qps.tile([P, CPC], F32, tag="pv", bufs=3, name="pv")
                        for k in range(KE):
                            nc.tensor.matmul(
                                pv[:], e_t[:, k, tt * P:(tt + 1) * P], wv_t[:, k, :],
                                start=(k == 0), stop=(k == KE - 1),
                            )
                        vst = qio.tile([P, CPC], F32R, tag="vst", bufs=4, name="vst")
                        nc.vector.tensor_add(vst[:], pv[:], bvbc_t[:])
                        nc.sync.dma_start(
                            v_d.ap()[tb * 512 + tt * P: tb * 512 + (tt + 1) * P, :],
                            vst[:])

            # ================= Phase A: causal attention =================
            # scoresT/ctxT per (batch, head), all transposed; softmax denom
            # via ones-matmul; mask added on the PE via identity-matmul
            # accumulation; sc emission pipelined 2 deep; the per-q-tile
            # normalization (copy/reciprocal/broadcast/mul) is deferred one
            # q-tile so the PE never waits on the DVE chain. ctx is shipped
            # through two half-batch AllToAlls; the batch-0 one fires while
            # batch-1 attention still runs.
            with (
                tc.tile_pool(name="aio", bufs=2) as aio,
                tc.tile_pool(name="asc", bufs=1) as asc,
                tc.tile_pool(name="aps", bufs=1, space="PSUM") as aps,
            ):
                pending = None

                def finalize(st):
                    b, hc, qt, ctx_ps, l_ps = st
                    l_sb = asc.tile([1, 512], F32, tag="lsb", bufs=2, name="l_sb")
                    nc.vector.tensor_copy(l_sb[:], l_ps[:])
                    r_sb = asc.tile([1, 512], F32R, tag="rsb", bufs=2, name="r_sb")
                    with nc.allow_low_precision(reason="f32r softmax denom"):
                        nc.vector.reciprocal(r_sb[:], l_sb[:])
                    rbc_ps = aps.tile([P, 512], F32, tag="sc", bufs=2, name="rbc_ps")
                    nc.tensor.matmul(rbc_ps[:], ones[0:1, :], r_sb[:],
                                     start=True, stop=True)
                    ctx_sb = asc.tile([P, 512], F32, tag="ctxsb", bufs=2,
                                      name="ctx_sb")
                    nc.vector.tensor_copy(ctx_sb[:], ctx_ps[:])
                    ctx_f = asc.tile([P, 512], F32R, tag="ctxf", bufs=2, name="ctx_f")
                    nc.vector.tensor_mul(ctx_f[:], ctx_sb[:], rbc_ps[:])
                    if b == 0:
                        nc.sync.dma_start(
                            a2a0in_d.ap()[2 * qt, hc * P:(hc + 1) * P, :],
                            ctx_f[:, 0:HB])
                        nc.sync.dma_start(
                            a2a0in_d.ap()[2 * qt + 1, hc * P:(hc + 1) * P, :],
                            ctx_f[:, HB:])
                    else:
                        dst = a2a1ain_d if hc == 0 else a2a1bin_d
                        nc.sync.dma_start(dst.ap()[2 * qt, :, :], ctx_f[:, 0:HB])
                        nc.sync.dma_start(dst.ap()[2 * qt + 1, :, :], ctx_f[:, HB:])

                for pi in range(4):
                    b = pi // 2
                    hc = pi % 2
                    q_t = aio.tile([P, S], F32R, tag="q", bufs=2, name="q_t")
                    nc.sync.dma_start(
                        q_t[:], qT_d.ap()[hc * P:(hc + 1) * P, b * S:(b + 1) * S])
                    k_t = aio.tile([P, S], F32R, tag="k", bufs=2, name="k_t")
                    nc.sync.dma_start(
                        k_t[:], kT_d.ap()[hc * P:(hc + 1) * P, b * S:(b + 1) * S])
                    v_t = aio.tile([P, 16, P], F32R, tag="v", bufs=2, name="v_t")
                    nc.sync.dma_start(
                        v_t[:],
                        v_d.ap()[b * S:(b + 1) * S, hc * P:(hc + 1) * P]
                        .rearrange("(j p) d -> p j d", p=P),
                    )
                    for qt in range(4):
                        nkb = 4 * qt + 4
                        ctx_ps = aps.tile([P, 512], F32, tag="ctx", bufs=2,
                                          name="ctx_ps")
                        l_ps = aps.tile([1, 512], F32, tag="l", bufs=2, name="l_ps")
                        ex_tiles = [None] * nkb
                        sc_cur = [None]

                        def emit_sc(kb, qt=qt, k_t=k_t, q_t=q_t,
                                    ex_tiles=ex_tiles, sc_cur=sc_cur):
                            # kb-blocks are processed in pairs sharing one
                            # 2-bank psum tile and a single wide Exp.
                            half = kb % 2
                            if half == 0:
                                sc_cur[0] = aps.tile([P, 2, 512], F32, tag="sc",
                                                     bufs=2, name="sc_ps")
                            sc_ps = sc_cur[0]
                            diag = kb >= 4 * qt
                            nc.tensor.matmul(
                                sc_ps[:, half, :], k_t[:, kb * P:(kb + 1) * P],
                                q_t[:, qt * 512:(qt + 1) * 512],
                                start=True, stop=not diag)
                            if diag:
                                nc.tensor.matmul(
                                    sc_ps[:, half, :], eye[:],
                                    mask_t[:, kb - 4 * qt, :],
                                    start=False, stop=True)
                            if half == 1:
                                ex = asc.tile([P, 2, 512], F32R, tag="ex", bufs=3,
                                              name="ex")
                                nc.scalar.activation(ex[:], sc_ps[:], Exp)
                                ex_tiles[kb - 1] = ex[:, 0, :]
                                ex_tiles[kb] = ex[:, 1, :]

                        for w in range(min(4, nkb)):
                            emit_sc(w)
                        for kb in range(nkb):
                            if kb + 4 < nkb:
                                emit_sc(kb + 4)
                            ex = ex_tiles[kb]
                            nc.tensor.matmul(ctx_ps[:], v_t[:, kb, :], ex,
                                             start=(kb == 0), stop=(kb == nkb - 1))
                            nc.tensor.matmul(l_ps[:], ones[:, 0:1], ex,
                                             start=(kb == 0), stop=(kb == nkb - 1))
                            ex_tiles[kb] = None
                        if pending is not None:
                            pb, phc, pqt = pending[0], pending[1], pending[2]
                            finalize(pending)
                            if (pb, phc, pqt) == (0, 1, 3):
                                # batch 0 fully written -> exchange it while
                                # batch-1 attention continues.
                                nc.gpsimd.collective_compute(
                                    "AllToAll", mybir.AluOpType.bypass,
                                    replica_groups=[list(range(N_CORES))],
                                    ins=[a2a0in_d.ap()], outs=[a2a0out_d.ap()],
                                )
                            elif (pb, phc, pqt) == (1, 0, 3):
                                # batch 1, head 0 written -> exchange during
                                # the last head's attention.
                                nc.gpsimd.collective_compute(
                                    "AllToAll", mybir.AluOpType.bypass,
                                    replica_groups=[list(range(N_CORES))],
                                    ins=[a2a1ain_d.ap()], outs=[a2a1aout_d.ap()],
                                )
                        pending = (b, hc, qt, ctx_ps, l_ps)
                finalize(pending)

            # ================= AllToAll: batch-1 head-1 ctx ================
            nc.gpsimd.collective_compute(
                "AllToAll", mybir.AluOpType.bypass,
                replica_groups=[list(range(N_CORES))],
                ins=[a2a1bin_d.ap()], outs=[a2a1bout_d.ap()],
            )

            # ====== Phase F: fc with full Wfc + residual + LN1 stats =======
            with (
                tc.tile_pool(name="fio", bufs=1) as fio,
                tc.tile_pool(name="fps", bufs=1, space="PSUM") as fps,
            ):
                x_t = fio.tile([P, KE, TBLK], F32R, name="x_t")
                ctxL = fio.tile([P, KE, TBLK], F32R, name="ctxL")
                nc.sync.dma_start(
                    ctxL[:, :, 0:HB],
                    a2a0out_d.ap().rearrange("r (c p) t -> p (r c) t", p=P))
                ctxL4 = ctxL[:].rearrange("p (r c) t -> p r c t", c=2)
                nc.sync.dma_start(
                    ctxL4[:, :, 0, HB:],
                    a2a1aout_d.ap().rearrange("r p t -> p r t"))
                nc.sync.dma_start(
                    ctxL4[:, :, 1, HB:],
                    a2a1bout_d.ap().rearrange("r p t -> p r t"))
                mu_ps = fps.tile([1, 512], F32, tag="ln1_mu", bufs=1, name="ln1_mu")
                sq_ps = fps.tile([1, 512], F32, tag="ln1_sq", bufs=1, name="ln1_sq")
                def fc_stats(nb):
                    nc.tensor.matmul(mu_ps[:], ones[:, 0:1], x_t[:, nb, :],
                                     start=(nb == 0), stop=(nb == 15))
                    sqk = fio.tile([P, 512], F32R, tag="sqk", bufs=3, name="sqk")
                    nc.vector.tensor_mul(sqk[:], x_t[:, nb, :], x_t[:, nb, :])
                    nc.tensor.matmul(sq_ps[:], ones[:, 0:1], sqk[:],
                                     start=(nb == 0), stop=(nb == 15))

                for nb in range(16):
                    wfc_t = fio.tile([P, KE, P], F32R, tag="wfc", bufs=3, name="wfc_t")
                    nc.scalar.dma_start(
                        wfc_t[:], wfc_d.ap()[nb].rearrange("p (k m) -> p k m", k=KE))
                    embres_t = fio.tile([P, TBLK], F32, tag="embres", bufs=2,
                                        name="embres_t")
                    nc.sync.dma_start(
                        embres_t[:], embres_d[:, nb * TBLK:(nb + 1) * TBLK])
                    pfc = fps.tile([P, 512], F32, tag="pfc", bufs=3, name="pfc")
                    for k in range(KE):
                        nc.tensor.matmul(pfc[:], wfc_t[:, k, :], ctxL[:, k, :],
                                         start=(k == 0), stop=(k == KE - 1))
                    nc.vector.scalar_tensor_tensor(
                        x_t[:, nb, :], pfc[:], bfc_t[:, nb:nb + 1],
                        embres_t[:], ADD, ADD)
                    if nb > 0:
                        fc_stats(nb - 1)
                fc_stats(15)
                _ln_finish(nc, fio, fps, x_t, ones, grows_d, 0, g1_t, be1_t,
                           eps_t, old_t, mu_ps, sq_ps, "ln1")

            # ================= Phase N: FFN =================
            with (
                tc.tile_pool(name="nw", bufs=1) as nw,
                tc.tile_pool(name="nps", bufs=1, space="PSUM") as nps,
            ):
                for hbg in range(4):
                    h_t = nw.tile([P, 16, TBLK], F32R, tag="h", bufs=1, name="h_t")
                    for hl in range(16):
                        hb = hbg * 16 + hl
                        w1_t = nw.tile([P, KE, P], F32R, tag="w1", bufs=4, name="w1_t")
                        nc.sync.dma_start(
                            w1_t[:], w1_d.ap()[hb].rearrange("p (k m) -> p k m", k=KE))
                        hps = nps.tile([P, 512], F32, tag="hps", bufs=3, name="hps")
                        for k in range(KE):
                            nc.tensor.matmul(hps[:], w1_t[:, k, :], old_t[:, k, :],
                                             start=(k == 0), stop=(k == KE - 1))
                        nc.scalar.activation(h_t[:, hl, :], hps[:], Gelu,
                                             bias=b1_t[:, hb:hb + 1])
                    for nb in range(16):
                        w2_t = nw.tile([P, 16, P], F32R, tag="w2", bufs=4, name="w2_t")
                        nc.scalar.dma_start(
                            w2_t[:],
                            w2_d.ap()[hbg, nb].rearrange("p (l m) -> p l m", l=16))
                        yps = nps.tile([P, 512], F32, tag="yps", bufs=3, name="yps")
                        for hl in range(16):
                            nc.tensor.matmul(yps[:], w2_t[:, hl, :], h_t[:, hl, :],
                                             start=(hl == 0), stop=(hl == 15))
                        if hbg == 0:
                            nc.vector.tensor_copy(y_sb[:, nb, :], yps[:])
                        else:
                            nc.vector.tensor_add(y_sb[:, nb, :], y_sb[:, nb, :], yps[:])

            # ================= Phase L2: residual + layernorm 2 ============
            with (
                tc.tile_pool(name="l2", bufs=1) as l2p,
                tc.tile_pool(name="l2ps", bufs=1, space="PSUM") as l2ps,
            ):
                x2_t = l2p.tile([P, KE, TBLK], F32R, name="x2_t")
                mu2_ps = l2ps.tile([1, 512], F32, tag="ln2_mu", bufs=1, name="ln2_mu")
                sq2_ps = l2ps.tile([1, 512], F32, tag="ln2_sq", bufs=1, name="ln2_sq")
                sq2_t = l2p.tile([P, KE, TBLK], F32R, name="sq2_t")
                for k in range(KE):
                    nc.vector.scalar_tensor_tensor(
                        x2_t[:, k, :], y_sb[:, k, :], b2_t[:, k:k + 1],
                        old_t[:, k, :], ADD, ADD)
                    nc.vector.tensor_mul(sq2_t[:, k, :], x2_t[:, k, :],
                                         x2_t[:, k, :])
                for k in range(KE):
                    nc.tensor.matmul(mu2_ps[:], ones[:, 0:1], x2_t[:, k, :],
                                     start=(k == 0), stop=(k == KE - 1))
                    nc.tensor.matmul(sq2_ps[:], ones[:, 0:1], sq2_t[:, k, :],
                                     start=(k == 0), stop=(k == KE - 1))
                out_sb = l2p.tile([P, KE, TBLK], F32, name="out_sb")
                _ln_finish(nc, l2p, l2ps, x2_t, ones, grows_d, 1, g2_t, be2_t,
                           eps_t, out_sb, mu2_ps, sq2_ps, "ln2",
                           chunk_done=lambda k: nc.sync.dma_start(
                               out_d.ap()[:, k, :], out_sb[:, k, :]))

    nc.compile()
    return nc


@functools.lru_cache(maxsize=1)
def _get_program():
    return _build_program()


def _pack_w(w):
    """[E_rows, M] -> [128, (E_rows/128)*M] with [p, k, m] layout."""
    e, m = w.shape
    return np.ascontiguousarray(
        w.reshape(e // P, P, m).transpose(1, 0, 2).reshape(P, -1))


def _pack_vec(v):
    """[n*128] -> [128, n] per-partition chunks."""
    return np.ascontiguousarray(v.reshape(-1, P).T)


def _prepare_in_maps(inputs):
    f32 = np.float32
    emb = np.asarray(inputs["embeddings"], f32).reshape(T, E)
    embT = np.ascontiguousarray(emb.T)
    scale = 1.0 / math.sqrt(HD)

    Wq = np.asarray(inputs["Wq"], f32)
    Wk = np.asarray(inputs["Wk"], f32)
    Wv = np.asarray(inputs["Wv"], f32)
    bq = np.asarray(inputs["bq"], f32)
    bk = np.asarray(inputs["bk"], f32)
    bv = np.asarray(inputs["bv"], f32)
    Wfc = np.asarray(inputs["Wfc"], f32)
    W1 = np.asarray(inputs["W1"], f32)
    W2 = np.asarray(inputs["W2"], f32)

    vecs = np.concatenate([
        _pack_vec(np.asarray(inputs[n], f32))
        for n in ("bfc", "g1", "be1", "b2", "g2", "be2")
    ], axis=1)  # [128, 6*KE]

    wfcp = np.ascontiguousarray(
        Wfc.reshape(KE, P, 16, P).transpose(2, 1, 0, 3).reshape(16, P, KE * P))
    w1p = np.ascontiguousarray(
        W1.reshape(KE, P, 64, P).transpose(2, 1, 0, 3).reshape(64, P, KE * P))
    w2p = np.ascontiguousarray(
        W2.reshape(4, 16, P, 16, P).transpose(0, 3, 2, 1, 4).reshape(4, 16, P, 16 * P))
    b1p = np.ascontiguousarray(np.asarray(inputs["b1"], f32).reshape(64, P).T)

    j = np.arange(P)[:, None, None]
    pp = np.arange(4)[None, :, None]
    cc = np.arange(512)[None, None, :]
    maskT = np.where(P * pp + j <= cc, 0.0, -30000.0).astype(f32).reshape(P, 4 * 512)
    onesblk = np.ones((P, P), f32)
    eyeblk = np.eye(P, dtype=f32)
    grows = np.concatenate([np.asarray(inputs["g1"], f32),
                            np.asarray(inputs["g2"], f32)]).reshape(1, 2 * KE * P)

    in_maps = []
    for c in range(N_CORES):
        sl = slice(CPC * c, CPC * (c + 1))
        bqs = (bq[sl] * scale).reshape(2, P).T
        bks = bk[sl].reshape(2, P).T
        in_maps.append({
            "embT": embT,
            "embres": np.ascontiguousarray(
                np.concatenate(
                    [embT[:, 256 * c:256 * (c + 1)],
                     embT[:, S + 256 * c:S + 256 * (c + 1)]], axis=1)
                .reshape(KE, P, TBLK).transpose(1, 0, 2).reshape(P, KE * TBLK)),
            "wq": _pack_w(Wq[:, sl] * scale),
            "wk": _pack_w(Wk[:, sl]),
            "wv": _pack_w(Wv[:, sl]),
            "bqk": np.ascontiguousarray(np.concatenate([bqs, bks], axis=1)),
            "bvbc": np.ascontiguousarray(np.broadcast_to(bv[sl], (P, CPC))),
            "wfc": wfcp,
            "vecs": vecs,
            "w1": w1p,
            "b1": b1p,
            "w2": w2p,
            "maskT": maskT,
            "onesblk": onesblk,
            "eyeblk": eyeblk,
            "grows": grows,
        })
    return in_maps


def kernel(**inputs) -> np.ndarray:
    nc = _get_program()
    in_maps = _prepare_in_maps(inputs)
    res = None
    last_err = None
    for attempt in range(3):
        try:
            res = run_bass_kernel_spmd(nc, in_maps, core_ids=list(range(N_CORES)))
            break
        except Exception as e:  # transient device/runtime hiccup: retry
            last_err = e
            import time as _time
            _time.sleep(3.0)
    if res is None:
        raise last_err
    out = np.empty((T, E), dtype=np.float32)
    for c in range(N_CORES):
        o = res.results[c]["outp"]          # [128, KE, 512] = [p, k, t]
        sl = o.transpose(1, 0, 2).reshape(E, TBLK)   # [E, 512]
        out[256 * c:256 * (c + 1)] = sl[:, 0:256].T
        out[S + 256 * c:S + 256 * (c + 1)] = sl[:, 256:].T
    return np.ascontiguousarray(out.reshape(B, S, E))


# revision 17
# speedup vs baseline: 1.0119x; 1.0119x over previous
"""Trainium2 Bass kernel for a single transformer decoder layer
(B=2, S=2048, E=2048, 16 heads, FFN 4x, causal attention, exact gelu,
two layernorms), distributed over 8 NeuronCores.

Sharding:
  - QKV + attention: tensor-parallel over heads (2 heads/core), zero comm.
  - One AllToAll exchanges ctx slices ([head-slice, all tokens] ->
    [all heads, 512-token slice]); each core then runs the fc projection
    with the full Wfc plus LN1 + FFN (full W1/W2) + LN2 on its own
    512-token slice. Host concatenates the 8 output slices.

Everything on-chip stays transposed ([feature, token]) so biases and
layernorm gains are per-partition ops and no transposes are needed.
Matmuls run in float32r (~13-bit mantissa, bf16 speed at N>=512).
"""
import functools
import math

import numpy as np

import concourse.bacc as bacc
import concourse.bass as bass
import concourse.mybir as mybir
import concourse.tile as tile
from concourse.bass_utils import run_bass_kernel_spmd

N_CORES = 8
P = 128
B, S, E = 2, 2048, 2048
T = B * S                   # 4096 tokens
NH, HD = 16, 128
FF = 4 * E                  # 8192
KE = E // P                 # 16 contraction chunks
CPC = 2 * HD                # 256 head-dim columns per core
TBLK = T // N_CORES         # 512 tokens per core after the all-to-all
EPS = 1e-5

F32 = mybir.dt.float32
F32R = mybir.dt.float32r

Identity = mybir.ActivationFunctionType.Identity
Copy = mybir.ActivationFunctionType.Copy
Exp = mybir.ActivationFunctionType.Exp
Gelu = mybir.ActivationFunctionType.Gelu
Sqrt = mybir.ActivationFunctionType.Sqrt
ADD = mybir.AluOpType.add
MULT = mybir.AluOpType.mult
SUB = mybir.AluOpType.subtract


def _ln_finish(nc, pool, psums, x_t, ones, grows_d, gi, g_t, be_t, eps_t,
               out_t, mu_ps, sq_ps, tag, chunk_done=None):
    """Finish a layernorm whose \u03a3x and \u03a3x\u00b2 already sit in mu_ps/sq_ps
    [1,512] psums. x_t [128, KE, 512] f32r -> out_t.
    Apply is 2 DVE passes/chunk: out = (x*g)*bcast(rstd) + be - g\u2297(\u03bc*rstd)."""
    grows = pool.tile([1, KE * P], F32R, tag=f"{tag}_grows", bufs=1,
                      name=f"{tag}_grows")
    nc.sync.dma_start(grows[:], grows_d[:, gi * KE * P:(gi + 1) * KE * P])
    mu_sb = pool.tile([1, 512], F32, tag=f"{tag}_musb", bufs=1, name=f"{tag}_musb")
    nc.scalar.activation(mu_sb[:], mu_ps[:], Copy, scale=1.0 / E)
    m2_sb = pool.tile([1, 512], F32, tag=f"{tag}_m2sb", bufs=1, name=f"{tag}_m2sb")
    nc.scalar.activation(m2_sb[:], sq_ps[:], Copy, scale=1.0 / E)
    var = pool.tile([1, 512], F32, tag=f"{tag}_var", bufs=1, name=f"{tag}_var")
    nc.vector.tensor_mul(var[:], mu_sb[:], mu_sb[:])
    nc.vector.tensor_sub(var[:], m2_sb[:], var[:])
    std = pool.tile([1, 512], F32, tag=f"{tag}_std", bufs=1, name=f"{tag}_std")
    nc.scalar.activation(std[:], var[:], Sqrt, bias=eps_t[:])
    rstd = pool.tile([1, 512], F32R, tag=f"{tag}_rstd", bufs=1, name=f"{tag}_rstd")
    with nc.allow_low_precision(reason="f32r rstd feeds f32r broadcast matmul"):
        nc.vector.reciprocal(rstd[:], std[:])
    msr = pool.tile([1, 512], F32R, tag=f"{tag}_msr", bufs=1, name=f"{tag}_msr")
    nc.vector.tensor_mul(msr[:], mu_sb[:], rstd[:])
    rbc = psums.tile([P, 512], F32, tag=f"{tag}_rbc", bufs=1, name=f"{tag}_rbc")
    nc.tensor.matmul(rbc[:], ones[0:1, :], rstd[:], start=True, stop=True)
    for k in range(KE):
        mbcg = psums.tile([P, 512], F32, tag=f"{tag}_mbcg", bufs=2,
                          name=f"{tag}_mbcg")
        nc.tensor.matmul(
            mbcg[:], grows[0:1, k * P:(k + 1) * P],
            msr[:], start=True, stop=True)
        t1 = pool.tile([P, 512], F32, tag=f"{tag}_t1", bufs=2, name=f"{tag}_t1")
        nc.vector.scalar_tensor_tensor(
            t1[:], x_t[:, k, :], g_t[:, k:k + 1], rbc[:], MULT, MULT)
        nc.vector.scalar_tensor_tensor(
            out_t[:, k, :], t1[:], be_t[:, k:k + 1], mbcg[:], ADD, SUB)
        if chunk_done is not None:
            chunk_done(k)
    # (grows row gi*KE+k holds g[k*128:(k+1)*128] so mbcg = g_e * (mu*rstd)_t)


def _build_program():
    nc = bacc.Bacc("TRN2", target_bir_lowering=False, debug=False,
                   num_devices=N_CORES)

    # ---- per-core external inputs ----
    embT_d = nc.dram_tensor("embT", [E, T], F32R, kind="ExternalInput")
    embres_d = nc.dram_tensor("embres", [P, KE * TBLK], F32, kind="ExternalInput")
    wq_d = nc.dram_tensor("wq", [P, KE * CPC], F32R, kind="ExternalInput")
    wk_d = nc.dram_tensor("wk", [P, KE * CPC], F32R, kind="ExternalInput")
    wv_d = nc.dram_tensor("wv", [P, KE * CPC], F32R, kind="ExternalInput")
    bqk_d = nc.dram_tensor("bqk", [P, 4], F32, kind="ExternalInput")  # bq|bk chunks
    bvbc_d = nc.dram_tensor("bvbc", [P, CPC], F32, kind="ExternalInput")
    wfc_d = nc.dram_tensor("wfc", [16, P, KE * P], F32R, kind="ExternalInput")
    vecs_d = nc.dram_tensor("vecs", [P, 6 * KE], F32, kind="ExternalInput")
    # vecs: [bfc | g1 | be1 | b2 | g2 | be2] each [P, KE]
    w1_d = nc.dram_tensor("w1", [64, P, KE * P], F32R, kind="ExternalInput")
    b1_d = nc.dram_tensor("b1", [P, 64], F32, kind="ExternalInput")
    w2_d = nc.dram_tensor("w2", [4, 16, P, 16 * P], F32R, kind="ExternalInput")
    mask_d = nc.dram_tensor("maskT", [P, 4 * 512], F32R, kind="ExternalInput")
    ones_d = nc.dram_tensor("onesblk", [P, P], F32R, kind="ExternalInput")
    eye_d = nc.dram_tensor("eyeblk", [P, P], F32R, kind="ExternalInput")
    grows_d = nc.dram_tensor("grows", [1, 2 * KE * P], F32R, kind="ExternalInput")

    out_d = nc.dram_tensor("outp", [P, KE, TBLK], F32, kind="ExternalOutput")

    # ---- internal DRAM ----
    qT_d = nc.dram_tensor("qT_i", [CPC, T], F32R, kind="Internal")
    kT_d = nc.dram_tensor("kT_i", [CPC, T], F32R, kind="Internal")
    v_d = nc.dram_tensor("v_i", [T, CPC], F32R, kind="Internal")
    HB = TBLK // 2   # 256-token half-block
    a2a0in_d = nc.dram_tensor("a2a0in_i", [N_CORES, CPC, HB], F32R, kind="Internal")
    a2a0out_d = nc.dram_tensor("a2a0out_i", [N_CORES, CPC, HB], F32R, kind="Internal")
    a2a1ain_d = nc.dram_tensor("a2a1ain_i", [N_CORES, P, HB], F32R, kind="Internal")
    a2a1aout_d = nc.dram_tensor("a2a1aout_i", [N_CORES, P, HB], F32R, kind="Internal")
    a2a1bin_d = nc.dram_tensor("a2a1bin_i", [N_CORES, P, HB], F32R, kind="Internal")
    a2a1bout_d = nc.dram_tensor("a2a1bout_i", [N_CORES, P, HB], F32R, kind="Internal")

    with tile.TileContext(nc) as tc:
        with (
            tc.tile_pool(name="const", bufs=1) as cpool,
            tc.tile_pool(name="persist", bufs=1) as ppool,
        ):
            ones = cpool.tile([P, P], F32R, name="ones")
            nc.sync.dma_start(ones[:], ones_d[:])
            eye = cpool.tile([P, P], F32R, name="eye")
            nc.sync.dma_start(eye[:], eye_d[:])
            mask_t = cpool.tile([P, 4, 512], F32R, name="mask_t")
            nc.sync.dma_start(mask_t[:], mask_d[:].rearrange("p (f t) -> p f t", f=4))
            bqk_t = cpool.tile([P, 4], F32, name="bqk_t")
            nc.sync.dma_start(bqk_t[:], bqk_d[:])
            bvbc_t = cpool.tile([P, CPC], F32, name="bvbc_t")
            nc.sync.dma_start(bvbc_t[:], bvbc_d[:])
            vecs_t = cpool.tile([P, 6, KE], F32, name="vecs_t")
            nc.sync.dma_start(vecs_t[:], vecs_d[:].rearrange("p (v k) -> p v k", v=6))
            b1_t = cpool.tile([P, 64], F32, name="b1_t")
            nc.sync.dma_start(b1_t[:], b1_d[:])
            eps_t = cpool.tile([1, 1], F32, name="eps_t")
            nc.vector.memset(eps_t[:], EPS)

            bfc_t = vecs_t[:, 0, :]
            g1_t = vecs_t[:, 1, :]
            be1_t = vecs_t[:, 2, :]
            b2_t = vecs_t[:, 3, :]
            g2_t = vecs_t[:, 4, :]
            be2_t = vecs_t[:, 5, :]

            old_t = ppool.tile([P, KE, TBLK], F32R, name="old_t")   # LN1 output
            y_sb = ppool.tile([P, KE, TBLK], F32, name="y_sb")      # FFN accum

            # ================= Phase Q: q/k/v projections =================
            with (
                tc.tile_pool(name="qw", bufs=1) as qw,
                tc.tile_pool(name="qio", bufs=2) as qio,
                tc.tile_pool(name="qps", bufs=1, space="PSUM") as qps,
            ):
                wq_t = qw.tile([P, KE, CPC], F32R, name="wq_t")
                nc.sync.dma_start(wq_t[:], wq_d[:].rearrange("p (k m) -> p k m", k=KE))
                wk_t = qw.tile([P, KE, CPC], F32R, name="wk_t")
                nc.sync.dma_start(wk_t[:], wk_d[:].rearrange("p (k m) -> p k m", k=KE))
                wv_t = qw.tile([P, KE, CPC], F32R, name="wv_t")
                nc.sync.dma_start(wv_t[:], wv_d[:].rearrange("p (k m) -> p k m", k=KE))

                for tb in range(8):
                    e_t = qio.tile([P, KE, 512], F32R, tag="emb", bufs=2, name="e_t")
                    nc.sync.dma_start(
                        e_t[:],
                        embT_d[:, tb * 512:(tb + 1) * 512]
                        .rearrange("(k p) t -> p k t", p=P),
                    )
                    for wi, (wt, dst) in enumerate(((wq_t, qT_d), (wk_t, kT_d))):
                        for hc in range(2):
                            pqk = qps.tile([P, 512], F32, tag="pqk", bufs=3, name="pqk")
                            for k in range(KE):
                                nc.tensor.matmul(
                                    pqk[:], wt[:, k, hc * P:(hc + 1) * P],
                                    e_t[:, k, :],
                                    start=(k == 0), stop=(k == KE - 1),
                                )
                            st = qio.tile([P, 512], F32R, tag="qkst", bufs=4, name="st")
                            nc.scalar.activation(st[:], pqk[:], Identity,
                                                 bias=bqk_t[:, 2 * wi + hc:2 * wi + hc + 1])
                            nc.sync.dma_start(
                                dst.ap()[hc * P:(hc + 1) * P, tb * 512:(tb + 1) * 512],
                                st[:])
                    for tt in range(4):
                        pv = qps.tile([P, CPC], F32, tag="pv", bufs=3, name="pv")
                        for k in range(KE):
                            nc.tensor.matmul(
                                pv[:], e_t[:, k, tt * P:(tt + 1) * P], wv_t[:, k, :],
                                start=(k == 0), stop=(k == KE - 1),
                            )
                        vst = qio.tile([P, CPC], F32R, tag="vst", bufs=4, name="vst")
                        nc.vector.tensor_add(vst[:], pv[:], bvbc_t[:])
                        nc.sync.dma_start(
                            v_d.ap()[tb * 512 + tt * P: tb * 512 + (tt + 1) * P, :],
                            vst[:])

            # ================= Phase A: causal attention =================
            # scoresT/ctxT per (batch, head), all transposed; softmax denom
            # via ones-matmul; mask added on the PE via identity-matmul
            # accumulation; sc emission pipelined 2 deep; the per-q-tile
            # normalization (copy/reciprocal/broadcast/mul) is deferred one
            # q-tile so the PE never waits on the DVE chain. ctx is shipped
            # through two half-batch AllToAlls; the batch-0 one fires while
            # batch-1 attention still runs.
            with (
                tc.tile_pool(name="aio", bufs=2) as aio,
                tc.tile_pool(name="asc", bufs=1) as asc,
                tc.tile_pool(name="aps", bufs=1, space="PSUM") as aps,
            ):
                pending = None

                def finalize(st):
                    b, hc, qt, ctx_ps, l_ps = st
                    l_sb = asc.tile([1, 512], F32, tag="lsb", bufs=2, name="l_sb")
                    nc.vector.tensor_copy(l_sb[:], l_ps[:])
                    r_sb = asc.tile([1, 512], F32R, tag="rsb", bufs=2, name="r_sb")
                    with nc.allow_low_precision(reason="f32r softmax denom"):
                        nc.vector.reciprocal(r_sb[:], l_sb[:])
                    rbc_ps = aps.tile([P, 512], F32, tag="sc", bufs=2, name="rbc_ps")
                    nc.tensor.matmul(rbc_ps[:], ones[0:1, :], r_sb[:],
                                     start=True, stop=True)
                    ctx_sb = asc.tile([P, 512], F32, tag="ctxsb", bufs=2,
                                      name="ctx_sb")
                    nc.vector.tensor_copy(ctx_sb[:], ctx_ps[:])
                    ctx_f = asc.tile([P, 512], F32R, tag="ctxf", bufs=2, name="ctx_f")
                    nc.vector.tensor_mul(ctx_f[:], ctx_sb[:], rbc_ps[:])
                    if b == 0:
                        nc.sync.dma_start(
                            a2a0in_d.ap()[2 * qt, hc * P:(hc + 1) * P, :],
                            ctx_f[:, 0:HB])
                        nc.sync.dma_start(
                            a2a0in_d.ap()[2 * qt + 1, hc * P:(hc + 1) * P, :],
                            ctx_f[:, HB:])
                    else:
                        dst = a2a1ain_d if hc == 0 else a2a1bin_d
                        nc.sync.dma_start(dst.ap()[2 * qt, :, :], ctx_f[:, 0:HB])
                        nc.sync.dma_start(dst.ap()[2 * qt + 1, :, :], ctx_f[:, HB:])

                for pi in range(4):
                    b = pi // 2
                    hc = pi % 2
                    q_t = aio.tile([P, S], F32R, tag="q", bufs=2, name="q_t")
                    nc.sync.dma_start(
                        q_t[:], qT_d.ap()[hc * P:(hc + 1) * P, b * S:(b + 1) * S])
                    k_t = aio.tile([P, S], F32R, tag="k", bufs=2, name="k_t")
                    nc.sync.dma_start(
                        k_t[:], kT_d.ap()[hc * P:(hc + 1) * P, b * S:(b + 1) * S])
                    v_t = aio.tile([P, 16, P], F32R, tag="v", bufs=2, name="v_t")
                    nc.sync.dma_start(
                        v_t[:],
                        v_d.ap()[b * S:(b + 1) * S, hc * P:(hc + 1) * P]
                        .rearrange("(j p) d -> p j d", p=P),
                    )
                    for qt in range(4):
                        nkb = 4 * qt + 4
                        ctx_ps = aps.tile([P, 512], F32, tag="ctx", bufs=2,
                                          name="ctx_ps")
                        l_ps = aps.tile([1, 512], F32, tag="l", bufs=2, name="l_ps")
                        ex_tiles = [None] * nkb
                        sc_cur = [None]

                        def emit_sc(kb, qt=qt, k_t=k_t, q_t=q_t,
                                    ex_tiles=ex_tiles, sc_cur=sc_cur):
                            # kb-blocks are processed in pairs sharing one
                            # 2-bank psum tile and a single wide Exp.
                            half = kb % 2
                            if half == 0:
                                sc_cur[0] = aps.tile([P, 2, 512], F32, tag="sc",
                                                     bufs=2, name="sc_ps")
                            sc_ps = sc_cur[0]
                            diag = kb >= 4 * qt
                            nc.tensor.matmul(
                                sc_ps[:, half, :], k_t[:, kb * P:(kb + 1) * P],
                                q_t[:, qt * 512:(qt + 1) * 512],
                                start=True, stop=not diag)
                            if diag:
                                nc.tensor.matmul(
                                    sc_ps[:, half, :], eye[:],
                                    mask_t[:, kb - 4 * qt, :],
                                    start=False, stop=True)
                            if half == 1:
                                ex = asc.tile([P, 2, 512], F32R, tag="ex", bufs=3,
                                              name="ex")
                                nc.scalar.activation(ex[:], sc_ps[:], Exp)
                                ex_tiles[kb - 1] = ex[:, 0, :]
                                ex_tiles[kb] = ex[:, 1, :]

                        for w in range(min(4, nkb)):
                            emit_sc(w)
                        for kb in range(nkb):
                            if kb + 4 < nkb:
                                emit_sc(kb + 4)
                            ex = ex_tiles[kb]
                            nc.tensor.matmul(ctx_ps[:], v_t[:, kb, :], ex,
                                             start=(kb == 0), stop=(kb == nkb - 1))
                            nc.tensor.matmul(l_ps[:], ones[:, 0:1], ex,
                                             start=(kb == 0), stop=(kb == nkb - 1))
                            ex_tiles[kb] = None
                        if pending is not None:
                            pb, phc, pqt = pending[0], pending[1], pending[2]
                            finalize(pending)
                            if (pb, phc, pqt) == (0, 1, 3):
                                # batch 0 fully written -> exchange it while
                                # batch-1 attention continues.
                                nc.gpsimd.collective_compute(
                                    "AllToAll", mybir.AluOpType.bypass,
                                    replica_groups=[list(range(N_CORES))],
                                    ins=[a2a0in_d.ap()], outs=[a2a0out_d.ap()],
                                )
                            elif (pb, phc, pqt) == (1, 0, 3):
                                # batch 1, head 0 written -> exchange during
                                # the last head's attention.
                                nc.gpsimd.collective_compute(
                                    "AllToAll", mybir.AluOpType.bypass,
                                    replica_groups=[list(range(N_CORES))],
                                    ins=[a2a1ain_d.ap()], outs=[a2a1aout_d.ap()],
                                )
                        pending = (b, hc, qt, ctx_ps, l_ps)
                finalize(pending)

            # ================= AllToAll: batch-1 head-1 ctx ================
            nc.gpsimd.collective_compute(
                "AllToAll", mybir.AluOpType.bypass,
                replica_groups=[list(range(N_CORES))],
                ins=[a2a1bin_d.ap()], outs=[a2a1bout_d.ap()],
            )

            # ====== Phase F: fc with full Wfc + residual + LN1 stats =======
            with (
                tc.tile_pool(name="fio", bufs=1) as fio,
                tc.tile_pool(name="fps", bufs=1, space="PSUM") as fps,
            ):
                x_t = fio.tile([P, KE, TBLK], F32R, name="x_t")
                ctxL = fio.tile([P, KE, TBLK], F32R, name="ctxL")
                nc.sync.dma_start(
                    ctxL[:, :, 0:HB],
                    a2a0out_d.ap().rearrange("r (c p) t -> p (r c) t", p=P))
                ctxL4 = ctxL[:].rearrange("p (r c) t -> p r c t", c=2)
                nc.sync.dma_start(
                    ctxL4[:, :, 0, HB:],
                    a2a1aout_d.ap().rearrange("r p t -> p r t"))
                nc.sync.dma_start(
                    ctxL4[:, :, 1, HB:],
                    a2a1bout_d.ap().rearrange("r p t -> p r t"))
                mu_ps = fps.tile([1, 512], F32, tag="ln1_mu", bufs=1, name="ln1_mu")
                sq_ps = fps.tile([1, 512], F32, tag="ln1_sq", bufs=1, name="ln1_sq")
                def fc_stats(nb):
                    nc.tensor.matmul(mu_ps[:], ones[:, 0:1], x_t[:, nb, :],
                                     start=(nb == 0), stop=(nb == 15))
                    sqk = fio.tile([P, 512], F32R, tag="sqk", bufs=3, name="sqk")
                    nc.vector.tensor_mul(sqk[:], x_t[:, nb, :], x_t[:, nb, :])
                    nc.tensor.matmul(sq_ps[:], ones[:, 0:1], sqk[:],
                                     start=(nb == 0), stop=(nb == 15))

                for nb in range(16):
                    wfc_t = fio.tile([P, KE, P], F32R, tag="wfc", bufs=3, name="wfc_t")
                    nc.sync.dma_start(
                        wfc_t[:], wfc_d.ap()[nb].rearrange("p (k m) -> p k m", k=KE))
                    embres_t = fio.tile([P, TBLK], F32, tag="embres", bufs=2,
                                        name="embres_t")
                    nc.sync.dma_start(
                        embres_t[:], embres_d[:, nb * TBLK:(nb + 1) * TBLK])
                    pfc = fps.tile([P, 512], F32, tag="pfc", bufs=3, name="pfc")
                    for k in range(KE):
                        nc.tensor.matmul(pfc[:], wfc_t[:, k, :], ctxL[:, k, :],
                                         start=(k == 0), stop=(k == KE - 1))
                    nc.vector.scalar_tensor_tensor(
                        x_t[:, nb, :], pfc[:], bfc_t[:, nb:nb + 1],
                        embres_t[:], ADD, ADD)
                    if nb > 0:
                        fc_stats(nb - 1)
                fc_stats(15)
                _ln_finish(nc, fio, fps, x_t, ones, grows_d, 0, g1_t, be1_t,
                           eps_t, old_t, mu_ps, sq_ps, "ln1")

            # ================= Phase N: FFN =================
            with (
                tc.tile_pool(name="nw", bufs=1) as nw,
                tc.tile_pool(name="nps", bufs=1, space="PSUM") as nps,
            ):
                for hbg in range(4):
                    h_t = nw.tile([P, 16, TBLK], F32R, tag="h", bufs=1, name="h_t")
                    for hl in range(16):
                        hb = hbg * 16 + hl
                        w1_t = nw.tile([P, KE, P], F32R, tag="w1", bufs=4, name="w1_t")
                        nc.sync.dma_start(
                            w1_t[:], w1_d.ap()[hb].rearrange("p (k m) -> p k m", k=KE))
                        hps = nps.tile([P, 512], F32, tag="hps", bufs=4, name="hps")
                        for k in range(KE):
                            nc.tensor.matmul(hps[:], w1_t[:, k, :], old_t[:, k, :],
                                             start=(k == 0), stop=(k == KE - 1))
                        nc.scalar.activation(h_t[:, hl, :], hps[:], Gelu,
                                             bias=b1_t[:, hb:hb + 1])
                    for nb in range(16):
                        w2_t = nw.tile([P, 16, P], F32R, tag="w2", bufs=4, name="w2_t")
                        nc.sync.dma_start(
                            w2_t[:],
                            w2_d.ap()[hbg, nb].rearrange("p (l m) -> p l m", l=16))
                        yps = nps.tile([P, 512], F32, tag="yps", bufs=4, name="yps")
                        for hl in range(16):
                            nc.tensor.matmul(yps[:], w2_t[:, hl, :], h_t[:, hl, :],
                                             start=(hl == 0), stop=(hl == 15))
                        if hbg == 0:
                            nc.vector.tensor_copy(y_sb[:, nb, :], yps[:])
                        else:
                            nc.vector.tensor_add(y_sb[:, nb, :], y_sb[:, nb, :], yps[:])

            # ================= Phase L2: residual + layernorm 2 ============
            with (
                tc.tile_pool(name="l2", bufs=1) as l2p,
                tc.tile_pool(name="l2ps", bufs=1, space="PSUM") as l2ps,
            ):
                x2_t = l2p.tile([P, KE, TBLK], F32R, name="x2_t")
                mu2_ps = l2ps.tile([1, 512], F32, tag="ln2_mu", bufs=1, name="ln2_mu")
                sq2_ps = l2ps.tile([1, 512], F32, tag="ln2_sq", bufs=1, name="ln2_sq")
                sq2_t = l2p.tile([P, KE, TBLK], F32R, name="sq2_t")
                for k in range(KE):
                    nc.vector.scalar_tensor_tensor(
                        x2_t[:, k, :], y_sb[:, k, :], b2_t[:, k:k + 1],
                        old_t[:, k, :], ADD, ADD)
                    nc.vector.tensor_mul(sq2_t[:, k, :], x2_t[:, k, :],
                                         x2_t[:, k, :])
                for k in range(KE):
                    nc.tensor.matmul(mu2_ps[:], ones[:, 0:1], x2_t[:, k, :],
                                     start=(k == 0), stop=(k == KE - 1))
                    nc.tensor.matmul(sq2_ps[:], ones[:, 0:1], sq2_t[:, k, :],
                                     start=(k == 0), stop=(k == KE - 1))
                out_sb = l2p.tile([P, KE, TBLK], F32, name="out_sb")
                _ln_finish(nc, l2p, l2ps, x2_t, ones, grows_d, 1, g2_t, be2_t,
                           eps_t, out_sb, mu2_ps, sq2_ps, "ln2",
                           chunk_done=lambda k: nc.sync.dma_start(
                               out_d.ap()[:, k, :], out_sb[:, k, :]))

    nc.compile()
    return nc


@functools.lru_cache(maxsize=1)
def _get_program():
    return _build_program()


def _pack_w(w):
    """[E_rows, M] -> [128, (E_rows/128)*M] with [p, k, m] layout."""
    e, m = w.shape
    return np.ascontiguousarray(
        w.reshape(e // P, P, m).transpose(1, 0, 2).reshape(P, -1))


def _pack_vec(v):
    """[n*128] -> [128, n] per-partition chunks."""
    return np.ascontiguousarray(v.reshape(-1, P).T)


def _prepare_in_maps(inputs):
    f32 = np.float32
    emb = np.asarray(inputs["embeddings"], f32).reshape(T, E)
    embT = np.ascontiguousarray(emb.T)
    scale = 1.0 / math.sqrt(HD)

    Wq = np.asarray(inputs["Wq"], f32)
    Wk = np.asarray(inputs["Wk"], f32)
    Wv = np.asarray(inputs["Wv"], f32)
    bq = np.asarray(inputs["bq"], f32)
    bk = np.asarray(inputs["bk"], f32)
    bv = np.asarray(inputs["bv"], f32)
    Wfc = np.asarray(inputs["Wfc"], f32)
    W1 = np.asarray(inputs["W1"], f32)
    W2 = np.asarray(inputs["W2"], f32)

    vecs = np.concatenate([
        _pack_vec(np.asarray(inputs[n], f32))
        for n in ("bfc", "g1", "be1", "b2", "g2", "be2")
    ], axis=1)  # [128, 6*KE]

    wfcp = np.ascontiguousarray(
        Wfc.reshape(KE, P, 16, P).transpose(2, 1, 0, 3).reshape(16, P, KE * P))
    w1p = np.ascontiguousarray(
        W1.reshape(KE, P, 64, P).transpose(2, 1, 0, 3).reshape(64, P, KE * P))
    w2p = np.ascontiguousarray(
        W2.reshape(4, 16, P, 16, P).transpose(0, 3, 2, 1, 4).reshape(4, 16, P, 16 * P))
    b1p = np.ascontiguousarray(np.asarray(inputs["b1"], f32).reshape(64, P).T)

    j = np.arange(P)[:, None, None]
    pp = np.arange(4)[None, :, None]
    cc = np.arange(512)[None, None, :]
    maskT = np.where(P * pp + j <= cc, 0.0, -30000.0).astype(f32).reshape(P, 4 * 512)
    onesblk = np.ones((P, P), f32)
    eyeblk = np.eye(P, dtype=f32)
    grows = np.concatenate([np.asarray(inputs["g1"], f32),
                            np.asarray(inputs["g2"], f32)]).reshape(1, 2 * KE * P)

    in_maps = []
    for c in range(N_CORES):
        sl = slice(CPC * c, CPC * (c + 1))
        bqs = (bq[sl] * scale).reshape(2, P).T
        bks = bk[sl].reshape(2, P).T
        in_maps.append({
            "embT": embT,
            "embres": np.ascontiguousarray(
                np.concatenate(
                    [embT[:, 256 * c:256 * (c + 1)],
                     embT[:, S + 256 * c:S + 256 * (c + 1)]], axis=1)
                .reshape(KE, P, TBLK).transpose(1, 0, 2).reshape(P, KE * TBLK)),
            "wq": _pack_w(Wq[:, sl] * scale),
            "wk": _pack_w(Wk[:, sl]),
            "wv": _pack_w(Wv[:, sl]),
            "bqk": np.ascontiguousarray(np.concatenate([bqs, bks], axis=1)),
            "bvbc": np.ascontiguousarray(np.broadcast_to(bv[sl], (P, CPC))),
            "wfc": wfcp,
            "vecs": vecs,
            "w1": w1p,
            "b1": b1p,
            "w2": w2p,
            "maskT": maskT,
            "onesblk": onesblk,
            "eyeblk": eyeblk,
            "grows": grows,
        })
    return in_maps


def kernel(**inputs) -> np.ndarray:
    nc = _get_program()
    in_maps = _prepare_in_maps(inputs)
    res = None
    last_err = None
    for attempt in range(3):
        try:
            res = run_bass_kernel_spmd(nc, in_maps, core_ids=list(range(N_CORES)))
            break
        except Exception as e:  # transient device/runtime hiccup: retry
            last_err = e
            import time as _time
            _time.sleep(3.0)
    if res is None:
        raise last_err
    out = np.empty((T, E), dtype=np.float32)
    for c in range(N_CORES):
        o = res.results[c]["outp"]          # [128, KE, 512] = [p, k, t]
        sl = o.transpose(1, 0, 2).reshape(E, TBLK)   # [E, 512]
        out[256 * c:256 * (c + 1)] = sl[:, 0:256].T
        out[S + 256 * c:S + 256 * (c + 1)] = sl[:, 256:].T
    return np.ascontiguousarray(out.reshape(B, S, E))
